# revision 61
# baseline (speedup 1.0000x reference)
"""MoH-MDTA attention kernel for Trainium2 (8 NeuronCores, data-parallel over batch).

The axon-tunneled devices make host<->device bytes the dominant cost (the
tunnel moves ~44 MB/s total, shared between directions), so the host/device
split is chosen to minimize tunnel traffic (~52 MB/call vs 316 MB for the
original design):
  - router softmax/top-2/gates are computed on HOST in exact fp32 (tiny GEMM;
    routing MUST use unquantized x -- top-2 selection from int8 x flips ~1% of
    pixels, an O(1) output error per flip). Gates ship packed 2 bytes/pixel
    (top-2 indices + larger-gate fraction, 33 KB/core) and are decoded to
    [8, N] bf16 on device via partition-broadcast + iota compare.
  - x is sent once per core as int8 with per-channel absmax scales
    (3.15 MB/core instead of 25 MB), dequantized to fp16 on device by one
    per-partition-column multiply per row-block.
  - output returns as int8 with per-(row, 128px-chunk) absmax scales in f16
    (3.2 MB/core), dequantized on host. Total quantization noise ~1.3% L2.
  - weights are content-hashed and cached device-resident across calls.
  - one single-core program per device, dispatched async core-by-core so host
    prep and all transfers pipeline over the tunnel; jitted callable + NEFF
    are cached so repeat calls do no compilation.

Per-core computation (one batch element, x [C=192, N=16384] layout):
  1. qkv 1x1 conv as fp16 matmuls, streamed over row-blocks with 1-row halos.
  2. depthwise 3x3 conv as 9 accumulating diagonal matmuls (bf16) on
     zero-padded row-block buffers (free-dim shifts only).
  3. v gated with host-computed gates (replicated per-head via stride-0 DMA).
  4. channel attention: per-head gram accumulation q@k^T via PE-transposed
     pixel tiles (head-pair groups of 96 rows include q/k norms on the diag),
     tiny softmax, attn @ v with gates pre-folded into v.
  5. final 1x1 proj conv, per-chunk int8 quantization, DMA out.
"""
import hashlib
import numpy as np
import ml_dtypes

C = 192
HEADS = 8
TOPK = 2
HD = C // HEADS  # 24

_CACHE = {}


def _build(H, W, RB, n_cores, dbg=False):
    import concourse.bacc as bacc
    import concourse.bass as bass
    import concourse.tile as tile
    import concourse.mybir as mybir
    from concourse.masks import make_identity
    from contextlib import ExitStack

    f32 = mybir.dt.float32
    f16 = mybir.dt.float16
    bf = mybir.dt.bfloat16
    i8 = mybir.dt.int8
    u8 = mybir.dt.uint8
    MULT = mybir.AluOpType.mult
    ADD = mybir.AluOpType.add
    SUB = mybir.AluOpType.subtract
    ISEQ = mybir.AluOpType.is_equal
    MAX = mybir.AluOpType.max
    RND = 12582912.0    # 1.5*2^23: (x + RND) - RND rounds x to nearest int
    Exp = mybir.ActivationFunctionType.Exp
    Sqrt = mybir.ActivationFunctionType.Sqrt
    AX = mybir.AxisListType.X

    N = H * W
    NB = H // RB
    assert H % RB == 0
    NT = RB * W // 128          # pixel-tiles per block (16 at full size)
    scale = HD ** -0.5

    nc = bacc.Bacc("TRN2", target_bir_lowering=False, debug=False,
                   num_devices=n_cores)

    # Coalesced per-core upload: rows 0..191 = x offset-encoded u8 (int8
    # quant + 128, per-channel absmax scales), rows 192..193 = gates packed
    # 2 bytes/pixel (row 192 = i1*8+i2 host-exact top-2 head indices, row
    # 193 = round((g1*TOPK - 1)*255); g2*TOPK = 2 - g1*TOPK exactly).
    xg_d = nc.dram_tensor("xg", [194, N], u8, kind="ExternalInput")
    xs_d = nc.dram_tensor("xs", [C, 1], f32, kind="ExternalInput")
    wA_d = nc.dram_tensor("wA", [C, 576], f16, kind="ExternalInput")
    dwd_d = nc.dram_tensor("dwd", [128, 45, 128], bf, kind="ExternalInput")
    pj_d = nc.dram_tensor("pj", [C, C], bf, kind="ExternalInput")
    NCH = N // 512
    # int8-style output: u8 quantized (round(po*127/absmax) + 128) with
    # per-(row, 128px-chunk) absmax scales shipped separately as f16
    outs_d = nc.dram_tensor("outs", [96, 2 * NCH * 4], f16, kind="ExternalOutput")
    outb_d = nc.dram_tensor("outb", [C, N], u8, kind="ExternalOutput")
    if dbg:
        dbg_v0 = nc.dram_tensor("dbg_v0", [96, N], f32, kind="ExternalOutput")
        dbg_qk0 = nc.dram_tensor("dbg_qk0", [96, N], f32, kind="ExternalOutput")
        dbg_gram = nc.dram_tensor("dbg_gram", [96, 384], f32, kind="ExternalOutput")
        dbg_bd = nc.dram_tensor("dbg_bd", [96, 192], f32, kind="ExternalOutput")
        dbg_pad0 = nc.dram_tensor("dbg_pad0", [128, (RB + 2) * (W + 2)], f32,
                                  kind="ExternalOutput")

    # conv output channel chunks: 4x128 qkv + 64 v-tail
    OCS = [(0, 128), (128, 128), (256, 128), (384, 128), (512, 64)]
    # dwconv channel chunks ( = pad buffers )
    DWS = [128, 128, 128, 128, 64]
    PADW = W + 2
    PADF = (RB + 2) * PADW

    with ExitStack() as top:
        tc = top.enter_context(tile.TileContext(nc))
        singles = top.enter_context(tc.tile_pool(name="singles", bufs=1))

        # --- resident constants ---
        wA0 = singles.tile([96, 576], f16)
        wA1 = singles.tile([96, 576], f16)
        nc.sync.dma_start(wA0[:], wA_d[0:96, :])
        nc.sync.dma_start(wA1[:], wA_d[96:192, :])
        dwd = singles.tile([128, 45, 128], bf)
        nc.sync.dma_start(dwd[:], dwd_d[:])
        xst = singles.tile([96, 2], f32)    # x dequant scales, rows 0..95/96..191
        nc.sync.dma_start(xst[:, 0:1], xs_d[0:96, :])
        nc.sync.dma_start(xst[:, 1:2], xs_d[96:192, :])
        iota8 = singles.tile([8, 1], f32)   # 0..7 column for gate-index decode
        iotai = singles.tile([8, 1], mybir.dt.int32)
        nc.gpsimd.iota(iotai[:], [[1, 1]], channel_multiplier=1)
        nc.vector.tensor_copy(iota8[:], iotai[:])
        ident = singles.tile([128, 128], f32)
        make_identity(nc, ident[:])
        identb = singles.tile([128, 128], bf)
        nc.vector.tensor_copy(identb[:], ident[:])
        pjt = singles.tile([96, 2, 2, 96], bf)   # [c-half, o-half][96c, 96o]
        for ch in range(2):
            for oh in range(2):
                nc.sync.dma_start(pjt[:, ch, oh, :],
                                  pj_d[96 * ch:96 * ch + 96, 96 * oh:96 * oh + 96])

        # --- resident accumulators / outputs of pass 1 ---
        v0 = singles.tile([96, N], bf)       # gated v, channels 0..95
        v1 = singles.tile([96, N], bf)       # gated v, channels 96..191
        gacc = singles.tile([96, 2, 192], f32)  # gram accumulators (4 groups)

        p1 = top.enter_context(ExitStack())
        xp = p1.enter_context(tc.tile_pool(name="xp", bufs=2))
        padp = p1.enter_context(tc.tile_pool(name="padp", bufs=1))
        qkp = p1.enter_context(tc.tile_pool(name="qkp", bufs=1))
        gep = p1.enter_context(tc.tile_pool(name="gep", bufs=1))
        stp = p1.enter_context(tc.tile_pool(name="stp", bufs=2))
        ps_conv = p1.enter_context(tc.tile_pool(name="ps_conv", bufs=1, space="PSUM"))
        ps_dw = p1.enter_context(tc.tile_pool(name="ps_dw", bufs=1, space="PSUM"))
        ps_tp = p1.enter_context(tc.tile_pool(name="ps_tp", bufs=1, space="PSUM"))
        ps_gr = p1.enter_context(tc.tile_pool(name="ps_gr", bufs=1, space="PSUM"))

        for b in range(NB):
            r0 = b * RB
            lo = max(r0 - 1, 0)              # first conv'd image row
            hi = min(r0 + RB + 1, H)         # one past last conv'd image row
            span = hi - lo                    # 16+1/2 rows incl halos
            spx = span * W

            # --- load x rows [lo, hi) as offset-u8, dequantize to f16 ---
            xi0 = xp.tile([96, (RB + 2) * W], u8, tag="xi0")
            xi1 = xp.tile([96, (RB + 2) * W], u8, tag="xi1")
            nc.sync.dma_start(xi0[:, 0:spx], xg_d[0:96, lo * W:hi * W])
            nc.sync.dma_start(xi1[:, 0:spx], xg_d[96:192, lo * W:hi * W])
            xb0 = xp.tile([96, (RB + 2) * W], f16, tag="xb0")
            xb1 = xp.tile([96, (RB + 2) * W], f16, tag="xb1")
            nc.vector.tensor_scalar(xb0[:, 0:spx], xi0[:, 0:spx],
                                    -128.0, xst[:, 0:1], op0=ADD, op1=MULT)
            nc.vector.tensor_scalar(xb1[:, 0:spx], xi1[:, 0:spx],
                                    -128.0, xst[:, 1:2], op0=ADD, op1=MULT)

            # --- pad buffers for dwconv input ---
            pads = [padp.tile([DWS[i], (RB + 2), PADW], bf, tag=f"pad{i}",
                              name=f"pad{i}") for i in range(5)]
            for i, pd in enumerate(pads):
                nc.vector.memset(pd[:, :, 0:1], 0)
                nc.vector.memset(pd[:, :, PADW - 1:PADW], 0)
                if b == 0:
                    nc.vector.memset(pd[:, 0:1, :], 0)
                if b == NB - 1:
                    nc.vector.memset(pd[:, RB + 1:RB + 2, :], 0)

            # --- conv1x1: chunks over the conv span ---
            chunks = []
            p0 = 0
            while p0 < spx:
                sz = min(512, spx - p0)
                chunks.append((p0, sz))
                p0 += sz
            for (p0, sz) in chunks:
                s_a = p0 // W + (1 if b == 0 else 0)   # pad-row of chunk start
                nrows = sz // W
                for oi, (ob, osz) in enumerate(OCS):
                    pc = ps_conv.tile([128, 512], f32, tag="pc")
                    mm = pc[0:osz, 0:sz]
                    nc.tensor.matmul(mm, wA0[:, ob:ob + osz], xb0[:, p0:p0 + sz],
                                     start=True, stop=False)
                    nc.tensor.matmul(mm, wA1[:, ob:ob + osz], xb1[:, p0:p0 + sz],
                                     start=False, stop=True)
                    src3 = pc[0:osz, 0:sz].rearrange("c (r w) -> c r w", w=W)
                    dst = pads[oi][:, s_a:s_a + nrows, 1:W + 1]
                    nc.any.tensor_copy(dst, src3)

            # --- host-packed gates: decode 2B/pixel -> [8, BW] bf16, replicate ---
            BW = RB * W
            gpk0 = gep.tile([1, BW], u8, tag="gpk0")
            gpk1 = gep.tile([1, BW], u8, tag="gpk1")
            nc.sync.dma_start(gpk0[:], xg_d[192:193, r0 * W:(r0 + RB) * W])
            nc.sync.dma_start(gpk1[:], xg_d[193:194, r0 * W:(r0 + RB) * W])
            bu0 = gep.tile([8, BW], u8, tag="bu0")
            bu1 = gep.tile([8, BW], u8, tag="bu1")
            nc.gpsimd.partition_broadcast(bu0[:], gpk0[:])
            nc.gpsimd.partition_broadcast(bu1[:], gpk1[:])
            bidx = gep.tile([8, BW], f32, tag="bidx")
            bg1 = gep.tile([8, BW], f32, tag="bg1")
            nc.vector.tensor_copy(bidx[:], bu0[:])
            nc.vector.tensor_scalar_mul(bg1[:], bu1[:], 1.0 / 255.0)
            # i1 = round(idx/8 - 0.4375); i2 = idx - 8*i1  (exact small ints)
            i1t = gep.tile([8, BW], f32, tag="i1t")
            nc.vector.tensor_scalar(i1t[:], bidx[:], 0.125, -0.4375,
                                    op0=MULT, op1=ADD)
            nc.vector.tensor_scalar(i1t[:], i1t[:], RND, -RND,
                                    op0=ADD, op1=ADD)
            i2t = gep.tile([8, BW], f32, tag="i2t")
            nc.vector.tensor_scalar(i2t[:], i1t[:], -8.0, None, op0=MULT)
            nc.vector.tensor_tensor(out=i2t[:], in0=i2t[:], in1=bidx[:], op=ADD)
            # e1/e2 in place of i1t/i2t; gates = e1 + e2 + (u/255)*(e1 - e2)
            nc.vector.tensor_scalar(i1t[:], i1t[:], iota8[:, 0:1], None, op0=ISEQ)
            nc.vector.tensor_scalar(i2t[:], i2t[:], iota8[:, 0:1], None, op0=ISEQ)
            nc.vector.tensor_tensor(out=bidx[:], in0=i1t[:], in1=i2t[:], op=SUB)
            nc.vector.tensor_tensor(out=bidx[:], in0=bidx[:], in1=bg1[:], op=MULT)
            nc.vector.tensor_tensor(out=bidx[:], in0=bidx[:], in1=i1t[:], op=ADD)
            gA = gep.tile([8, BW], bf, tag="gA")
            nc.vector.tensor_tensor(out=gA[:], in0=bidx[:], in1=i2t[:], op=ADD)
            gx0 = gep.tile([96, BW], bf, tag="gx0")   # heads 0..3 x24
            gx1 = gep.tile([96, BW], bf, tag="gx1")   # heads 4..7 x24
            s0 = bass.AP(tensor=gA.tensor, offset=gA[:].offset,
                         ap=[[BW, 4], [0, 24], [1, BW]])
            s1 = bass.AP(tensor=gA.tensor, offset=gA[4:8, :].offset,
                         ap=[[BW, 4], [0, 24], [1, BW]])
            nc.sync.dma_start(gx0[:], s0)
            nc.sync.dma_start(gx1[:], s1)

            # --- depthwise conv 3x3 + v gating ---
            qk = [qkp.tile([96, RB * W], bf, tag=f"qk{g}", name=f"qk{g}")
                  for g in range(4)]
            nch = RB * W // 512
            for ci in range(5):
                csz = DWS[ci]
                for u in range(nch):
                    pd = ps_dw.tile([128, 512], f32, tag="pd")
                    y0 = (u * 512) // W          # interior row offset 0..RB-1
                    nr = 512 // W
                    for t in range(9):
                        dy, dx = t // 3 - 1, t % 3 - 1
                        rhs = pads[ci][:, y0 + 1 + dy:y0 + 1 + dy + nr,
                                       1 + dx:1 + dx + W]
                        nc.tensor.matmul(
                            pd[0:csz, :].rearrange("c (r w) -> c r w", w=W),
                            dwd[0:csz, 5 * t + ci, 0:csz], rhs,
                            start=(t == 0), stop=(t == 8))
                    # NOTE: SBUF operands must start at partition {0,32,64,96}
                    # with span <= {128,32,64,32}; PSUM sources are exempt.
                    sl = slice(u * 512, (u + 1) * 512)
                    glob = slice(r0 * W + u * 512, r0 * W + (u + 1) * 512)
                    if ci == 0:
                        nc.any.tensor_copy(qk[0][0:96, sl], pd[0:96, :])
                        nc.any.tensor_copy(qk[1][0:32, sl], pd[96:128, :])
                    elif ci == 1:
                        nc.any.tensor_copy(qk[1][32:64, sl], pd[0:32, :])
                        nc.any.tensor_copy(qk[1][64:96, sl], pd[32:64, :])
                        nc.any.tensor_copy(qk[2][0:64, sl], pd[64:128, :])
                    elif ci == 2:
                        nc.any.tensor_copy(qk[2][64:96, sl], pd[0:32, :])
                        nc.any.tensor_copy(qk[3][0:32, sl], pd[32:64, :])
                        nc.any.tensor_copy(qk[3][32:64, sl], pd[64:96, :])
                        nc.any.tensor_copy(qk[3][64:96, sl], pd[96:128, :])
                    elif ci == 3:
                        nc.vector.tensor_tensor(out=v0[:, glob], in0=pd[0:96, :],
                                                in1=gx0[:, sl], op=MULT)
                        nc.vector.tensor_tensor(out=v1[0:32, glob],
                                                in0=pd[96:128, :],
                                                in1=gx1[0:32, sl], op=MULT)
                    else:
                        nc.vector.tensor_tensor(out=v1[32:64, glob],
                                                in0=pd[0:32, :],
                                                in1=gx1[32:64, sl], op=MULT)
                        nc.vector.tensor_tensor(out=v1[64:96, glob],
                                                in0=pd[32:64, :],
                                                in1=gx1[64:96, sl], op=MULT)

            # --- q/k pixel-tile transposes + gram accumulation ---
            grp = [ps_gr.tile([96, 96], f32, tag=f"gr{g}", name=f"gr{g}")
                   for g in range(4)]
            for j in range(NT):
                st = stp.tile([128, 4, 4, 24], bf, tag="st")  # [p, gp, slot, hd]
                for g in range(4):
                    tq = ps_tp.tile([128, 96], bf, tag="tq")
                    nc.tensor.transpose(tq[:], qk[g][:, j * 128:(j + 1) * 128],
                                        identb[0:96, 0:96])
                    src = tq[:].rearrange("p (a b h) -> p a b h", a=2, b=2, h=24)
                    if g == 0:
                        nc.any.tensor_copy(st[:, 0:2, 0:2, :], src)
                    elif g == 1:
                        nc.any.tensor_copy(st[:, 2:4, 0:2, :], src)
                    elif g == 2:
                        nc.any.tensor_copy(st[:, 0:2, 2:4, :], src)
                    else:
                        nc.any.tensor_copy(st[:, 2:4, 2:4, :], src)
                for gp in range(4):
                    lhs = st[:, gp, :, :].rearrange("p a b -> p (a b)")
                    nc.tensor.matmul(grp[gp], lhs, lhs,
                                     start=(j == 0), stop=(j == NT - 1))
            if dbg == 2 and b == 0:
                dp0 = qkp.tile([128, (RB + 2) * PADW], f32, tag="dp0")
                nc.vector.tensor_copy(dp0[:], pads[0][:].rearrange("c a b -> c (a b)"))
                nc.sync.dma_start(dbg_pad0[:], dp0[:])
            if dbg == 2:
                dv0 = qkp.tile([96, RB * W], f32, tag="dv0")
                nc.vector.tensor_copy(dv0[:], v0[:, r0 * W:(r0 + RB) * W])
                nc.sync.dma_start(dbg_v0[:, r0 * W:(r0 + RB) * W], dv0[:])
                dqk = qkp.tile([96, RB * W], f32, tag="dqk")
                nc.vector.tensor_copy(dqk[:], qk[0][:, 0:RB * W])
                nc.sync.dma_start(dbg_qk0[:, r0 * W:(r0 + RB) * W], dqk[:])
            for gp in range(4):
                dstg = gacc[:, gp // 2, (gp % 2) * 96:(gp % 2) * 96 + 96]
                if b == 0:
                    nc.any.tensor_copy(dstg, grp[gp])
                else:
                    nc.vector.tensor_tensor(out=dstg, in0=dstg, in1=grp[gp], op=ADD)
        p1.close()

        # ===== pass 2: attention matrices =====
        p2 = top.enter_context(ExitStack())
        smp = p2.enter_context(tc.tile_pool(name="smp", bufs=1))
        dramp = p2.enter_context(tc.tile_pool(name="dramp", bufs=1, space="DRAM"))
        # assemble block-diag attn in DRAM (partition-offset bf16 SBUF DMA
        # writes drop elements on HW), then load+convert once
        bd_dram = dramp.tile([96, 2, 96], f32)
        zst = smp.tile([96, 2, 96], f32, name="zst")
        nc.vector.memset(zst[:], 0)
        nc.sync.dma_start(bd_dram[:], zst[:])

        bd = [singles.tile([96, 96], bf, name="bd0"),
              singles.tile([96, 96], bf, name="bd1")]
        nc.vector.memset(bd[0][:], 0)
        nc.vector.memset(bd[1][:], 0)

        rinv = smp.tile([96, 4], f32)
        for gp in range(4):
            G = gacc[:, gp // 2, (gp % 2) * 96:(gp % 2) * 96 + 96]
            dt_ = smp.tile([96, 96], f32, tag="dt_")
            nc.vector.tensor_tensor(out=dt_[:], in0=G, in1=ident[0:96, 0:96],
                                    op=MULT)
            ssq = smp.tile([96, 1], f32, tag="ssq")
            nc.vector.tensor_reduce(ssq[:], dt_[:], axis=AX, op=ADD)
            nc.scalar.activation(ssq[:], ssq[:], Sqrt)
            nc.vector.tensor_scalar_max(ssq[:], ssq[:], 1e-12)
            nc.vector.reciprocal(rinv[:, gp:gp + 1], ssq[:])

        for gp in range(4):
            G = gacc[:, gp // 2, (gp % 2) * 96:(gp % 2) * 96 + 96]
            for m in range(2):
                h = 2 * gp + m
                # 24-row-aligned slices are illegal SBUF operands -> stage
                # through SBUF->SBUF DMA into partition-0-based tiles.
                gblk = smp.tile([24, 24], f32, tag="gblk")
                nc.sync.dma_start(gblk[:],
                                  G[24 * m:24 * m + 24, 48 + 24 * m:72 + 24 * m])
                rq = smp.tile([24, 1], f32, tag="rq")
                nc.sync.dma_start(rq[:], rinv[24 * m:24 * m + 24, gp:gp + 1])
                # k-norm column -> row via 32x32 DVE transpose
                zt = smp.tile([32, 32], f32, tag="zt")
                nc.vector.memset(zt[:], 0)
                nc.sync.dma_start(zt[0:24, 0:1],
                                  rinv[48 + 24 * m:72 + 24 * m, gp:gp + 1])
                ztt = smp.tile([32, 32], f32, tag="ztt")
                nc.vector.transpose(ztt[:], zt[:])
                O = smp.tile([24, 24], f32, tag="O")
                nc.gpsimd.partition_broadcast(O[:], ztt[0:1, 0:24])
                nc.vector.tensor_scalar(O[:], O[:], rq[:],
                                        float(scale), op0=MULT, op1=MULT)
                al32 = smp.tile([32, 32], f32, tag="al32")
                nc.vector.memset(al32[:], 0)
                al = al32[0:24, 0:24]
                nc.vector.tensor_tensor(out=al, in0=gblk[:], in1=O[:], op=MULT)
                negm = smp.tile([24, 1], f32, tag="negm")
                nc.vector.tensor_reduce(negm[:], al, axis=AX,
                                        op=mybir.AluOpType.max, negate=True)
                den = smp.tile([24, 1], f32, tag="den")
                nc.scalar.activation(al, al, Exp, bias=negm[:],
                                     accum_out=den[:])
                rden = smp.tile([24, 1], f32, tag="rden")
                nc.vector.reciprocal(rden[:], den[:])
                nc.vector.tensor_scalar(al, al, rden[:], None, op0=MULT)
                patv = smp.tile([32, 32], f32, tag="patv")
                nc.vector.transpose(patv[:], al32[:])
                sa = smp.tile([24, 24], f32, tag="sa")
                nc.any.tensor_copy(sa[:], patv[0:24, 0:24])
                hh = h % 4
                nc.sync.dma_start(bd_dram[24 * hh:24 * hh + 24, h // 4,
                                          24 * hh:24 * hh + 24], sa[:])
        bdf = smp.tile([96, 2, 96], f32, name="bdf")
        nc.sync.dma_start(bdf[:], bd_dram[:])
        nc.any.tensor_copy(bd[0][:], bdf[:, 0, :])
        nc.any.tensor_copy(bd[1][:], bdf[:, 1, :])
        if dbg:
            nc.sync.dma_start(dbg_gram[:], gacc[:].rearrange("p a b -> p (a b)"))
            dbd = smp.tile([96, 192], f32, name="dbd")
            nc.vector.tensor_copy(dbd[:, 0:96], bd[0][:])
            nc.vector.tensor_copy(dbd[:, 96:192], bd[1][:])
            nc.sync.dma_start(dbg_bd[:], dbd[:])
        p2.close()

        # ===== pass 3: attn @ v_gated, proj, int8 quantize, out =====
        p3 = top.enter_context(ExitStack())
        op_ = p3.enter_context(tc.tile_pool(name="op_", bufs=3))
        oqp = p3.enter_context(tc.tile_pool(name="oqp", bufs=1))
        ps3 = p3.enter_context(tc.tile_pool(name="ps3", bufs=2, space="PSUM"))
        q8 = [oqp.tile([96, N], u8, name="q80"),
              oqp.tile([96, N], u8, name="q81")]
        scol = oqp.tile([96, 2, NCH, 4], f32)
        for u in range(NCH):
            sl = slice(u * 512, (u + 1) * 512)
            avs = []
            for half in range(2):
                pav = ps3.tile([96, 512], f32, tag=f"pav{half}")
                nc.tensor.matmul(pav[:], bd[half][:], (v0 if half == 0 else v1)[:, sl],
                                 start=True, stop=True)
                av = op_.tile([96, 512], bf, tag=f"av{half}")
                nc.any.tensor_copy(av[:], pav[:])
                avs.append(av)
            for oh in range(2):
                po = ps3.tile([96, 512], f32, tag=f"po{oh}")
                nc.tensor.matmul(po[:], pjt[:, 0, oh, :], avs[0][:],
                                 start=True, stop=False)
                nc.tensor.matmul(po[:], pjt[:, 1, oh, :], avs[1][:],
                                 start=False, stop=True)
                po3 = po[:].rearrange("c (a b) -> c a b", b=128)
                av = scol[:, oh, u, :]                            # [96, 4]
                av3 = av.rearrange("c (a o) -> c a o", o=1)
                nc.vector.tensor_reduce(av3, po3, axis=AX, op=MAX,
                                        apply_absolute_value=True)
                nc.vector.tensor_scalar_max(av, av, 1e-30)
                rcl = op_.tile([96, 4], f32, tag=f"rcl{oh}")
                nc.vector.reciprocal(rcl[:], av)
                nc.vector.tensor_scalar_mul(rcl[:], rcl[:], 127.0)
                rclb = rcl[:].rearrange("c (a o) -> c a o", o=1) \
                             .to_broadcast([96, 4, 128])
                qf = op_.tile([96, 512], f32, tag=f"qf{oh}")
                qf3 = qf[:].rearrange("c (a b) -> c a b", b=128)
                nc.vector.tensor_tensor(out=qf3, in0=po3, in1=rclb, op=MULT)
                # round to nearest int, then offset by +128 into u8 range
                nc.vector.tensor_scalar(qf[:], qf[:], RND, -(RND - 128.0),
                                        op0=ADD, op1=ADD)
                nc.any.tensor_copy(q8[oh][:, sl], qf[:])
        scol16 = op_.tile([96, 2 * NCH * 4], f16, name="scol16")
        nc.vector.tensor_copy(scol16[:], scol[:].rearrange("p a b c -> p (a b c)"))
        nc.sync.dma_start(outs_d[:], scol16[:])
        nc.sync.dma_start(outb_d[0:96, :], q8[0][:])
        nc.sync.dma_start(outb_d[96:192, :], q8[1][:])
        p3.close()

    nc.finalize()
    return nc


def _host_prep(qkv_w, dw_w, proj_w):
    wA = np.ascontiguousarray(qkv_w.T).astype(np.float16)  # [192, 576]
    w9 = dw_w.reshape(3 * C, 9).astype(np.float32)
    dwd = np.zeros((128, 45, 128), dtype=ml_dtypes.bfloat16)
    DWS = [128, 128, 128, 128, 64]
    for t in range(9):
        for i in range(5):
            base = sum(DWS[:i])
            csz = DWS[i]
            m = np.zeros((128, 128), np.float32)
            np.fill_diagonal(m[:csz, :csz], w9[base:base + csz, t])
            dwd[:, 5 * t + i, :] = m.astype(ml_dtypes.bfloat16)
    pj = np.ascontiguousarray(proj_w.T).astype(ml_dtypes.bfloat16)  # [192c, 192o]
    return wA, dwd, pj


def _host_gates(x2, rw):
    """Exact fp32 router on host: x2 [B, C, N], rw [8, C] -> packed gates
    [B, 2, N] uint8: row 0 = i1*8+i2 (top-2 head indices), row 1 =
    round((g1*TOPK - 1)*255) with g1 the larger renormalized gate."""
    B = x2.shape[0]
    N = x2.shape[2]
    logits = np.matmul(rw[None].astype(np.float32), x2)          # [B, 8, N]
    m = logits.max(axis=1, keepdims=True)
    p = np.exp(logits - m)
    p /= p.sum(axis=1, keepdims=True)
    i1 = p.argmax(axis=1)                                        # [B, N]
    p1 = np.take_along_axis(p, i1[:, None], axis=1)[:, 0]
    np.put_along_axis(p, i1[:, None], -1.0, axis=1)
    i2 = p.argmax(axis=1)
    p2 = np.take_along_axis(p, i2[:, None], axis=1)[:, 0]
    g1t = (float(TOPK) * p1) / np.maximum(p1 + p2, 1.1920929e-7)  # in [1, 2]
    out = np.empty((B, 2, N), np.uint8)
    out[:, 0] = (i1 * 8 + i2).astype(np.uint8)
    out[:, 1] = np.rint((g1t - 1.0) * 255.0).astype(np.uint8)
    return out


class _State:
    pass


def _get_state(H, W, B):
    key = (H, W, B)
    if key in _CACHE:
        return _CACHE[key]
    import jax
    import jax.numpy as jnp
    import concourse.mybir as mybir
    from concourse.bass2jax import (_bass_exec_p, partition_id_tensor,
                                    install_neuronx_cc_hook)

    install_neuronx_cc_hook()
    nc = _build(H, W, 16, 1)

    partition_name = (nc.partition_id_tensor.name
                      if nc.partition_id_tensor else None)
    in_names, out_names, out_avals = [], [], []
    for alloc in nc.m.functions[0].allocations:
        if not isinstance(alloc, mybir.MemoryLocationSet):
            continue
        name = alloc.memorylocations[0].name
        if alloc.kind == "ExternalInput":
            if name != partition_name:
                in_names.append(name)
        elif alloc.kind == "ExternalOutput":
            out_names.append(name)
            out_avals.append(jax.core.ShapedArray(
                tuple(alloc.tensor_shape), mybir.dt.np(alloc.dtype)))
    n_params = len(in_names)
    all_names = tuple(in_names + out_names +
                      ([partition_name] if partition_name else []))
    donate = tuple(range(n_params, n_params + len(out_names)))

    def _body(*args):
        operands = list(args)
        if partition_name is not None:
            operands.append(partition_id_tensor())
        return tuple(_bass_exec_p.bind(
            *operands,
            out_avals=tuple(out_avals),
            in_names=all_names,
            out_names=tuple(out_names),
            lowering_input_output_aliases=(),
            sim_require_finite=True,
            sim_require_nnan=True,
            nc=nc,
        ))

    st = _State()
    st.nc = nc
    st.in_names = in_names
    st.out_names = out_names
    st.out_avals = out_avals
    st.jit_body = jax.jit(_body, donate_argnums=donate, keep_unused=True)
    st.devices = jax.devices()[:B]
    # donated output scratch buffers, one per device; replaced by each call's
    # result (the kernel writes every element of out, so no pre-zero needed)
    zeros = [np.zeros(tuple(a.shape), a.dtype) for a in out_avals]
    st.scratch = [[jax.device_put(z, d) for z in zeros] for d in st.devices]
    st.weights_fp = None
    st.wdev = None
    _CACHE[key] = st
    return st


def kernel(x, qkv_w, dw_w, proj_w, router_main_w, router_aux_w, task_id):
    import jax

    x = np.asarray(x, np.float32)
    B, c, H, W = x.shape
    assert c == C
    N = H * W
    tid = int(np.asarray(task_id))
    rw = np.asarray(router_main_w if tid == 0 else router_aux_w, np.float32)

    st = _get_state(H, W, B)

    # --- weights: content-hashed, uploaded to each device only on change
    # (rw excluded: the router runs on host, device weights don't use it) ---
    hsh = hashlib.blake2b(digest_size=16)
    for a in (qkv_w, dw_w, proj_w):
        hsh.update(np.ascontiguousarray(a, np.float32).tobytes())
    wfp = hsh.digest()
    if st.weights_fp != wfp:
        wA, dwd, pj = _host_prep(np.asarray(qkv_w, np.float32),
                                 np.asarray(dw_w, np.float32),
                                 np.asarray(proj_w, np.float32))
        wmap = {"wA": wA, "dwd": dwd, "pj": pj}
        st.wdev = [{k: jax.device_put(v, d) for k, v in wmap.items()}
                   for d in st.devices]
        st.weights_fp = wfp

    x2 = x.reshape(B, C, N)

    # --- dispatch per core, interleaving host conversion with the wire so
    # transfers start immediately; h2d / exec / d2h pipeline over the tunnel ---
    gates = _host_gates(x2, rw)                               # [B, 2, N] u8
    results = []
    for b in range(B):
        dev = st.devices[b]
        am = np.abs(x2[b]).max(axis=1, keepdims=True)         # [C, 1]
        qs = 127.0 / np.maximum(am, 1e-30)
        xg = np.empty((C + 2, N), np.uint8)
        t = x2[b] * qs
        np.rint(t, out=t)
        t += 128.0
        xg[0:C] = t.astype(np.uint8)
        xg[C:C + 2] = gates[b]
        per = {"xg": jax.device_put(xg, dev),
               "xs": jax.device_put((am / 127.0).astype(np.float32), dev)}
        ops = [per[name] if name in per else st.wdev[b][name]
               for name in st.in_names]
        res = st.jit_body(*ops, *st.scratch[b])
        for r in res:
            r.copy_to_host_async()
        results.append(res)

    oi = st.out_names.index("outb")
    si = st.out_names.index("outs")
    NQ = N // 128
    out = np.empty((B, C, N), np.float32)
    for b in range(B):
        s = np.asarray(results[b][si]).astype(np.float32)     # [96, 2*NQ] f16
        sc = (s.reshape(96, 2, NQ).transpose(1, 0, 2).reshape(C, NQ)
              * (1.0 / 127.0))
        q = np.asarray(results[b][oi]).astype(np.float32)     # [C, N] u8
        q -= 128.0
        np.multiply(q.reshape(C, NQ, 128), sc[:, :, None],
                    out=out[b].reshape(C, NQ, 128))
    st.scratch = [list(res) for res in results]
    return out.reshape(B, C, H, W)


# revision 62
# speedup vs baseline: 1.0826x; 1.0826x over previous
"""MoH-MDTA attention kernel for Trainium2 (8 NeuronCores, data-parallel over batch).

The axon-tunneled devices make host<->device bytes the dominant cost (the
tunnel moves ~44 MB/s total, shared between directions), so the host/device
split is chosen to minimize tunnel traffic (~52 MB/call vs 316 MB for the
original design):
  - router softmax/top-2/gates are computed on HOST in exact fp32 (tiny GEMM;
    routing MUST use unquantized x -- top-2 selection from int8 x flips ~1% of
    pixels, an O(1) output error per flip). Gates ship packed 2 bytes/pixel
    (top-2 indices + larger-gate fraction, 33 KB/core) and are decoded to
    [8, N] bf16 on device via partition-broadcast + iota compare.
  - x is sent once per core as int8 with per-channel absmax scales
    (3.15 MB/core instead of 25 MB), dequantized to fp16 on device by one
    per-partition-column multiply per row-block.
  - output returns as int8 with per-(row, 128px-chunk) absmax scales in f16
    (3.2 MB/core), dequantized on host. Total quantization noise ~1.3% L2.
  - weights are content-hashed and cached device-resident across calls.
  - one single-core program per device, dispatched async core-by-core so host
    prep and all transfers pipeline over the tunnel; jitted callable + NEFF
    are cached so repeat calls do no compilation.

Per-core computation (one batch element, x [C=192, N=16384] layout):
  1. qkv 1x1 conv as fp16 matmuls, streamed over row-blocks with 1-row halos.
  2. depthwise 3x3 conv as 9 accumulating diagonal matmuls (bf16) on
     zero-padded row-block buffers (free-dim shifts only).
  3. v gated with host-computed gates (replicated per-head via stride-0 DMA).
  4. channel attention: per-head gram accumulation q@k^T via PE-transposed
     pixel tiles (head-pair groups of 96 rows include q/k norms on the diag),
     tiny softmax, attn @ v with gates pre-folded into v.
  5. final 1x1 proj conv, per-chunk int8 quantization, DMA out.
"""
import hashlib
import numpy as np
import ml_dtypes

C = 192
HEADS = 8
TOPK = 2
HD = C // HEADS  # 24

_CACHE = {}


def _build(H, W, RB, n_cores, dbg=False):
    import concourse.bacc as bacc
    import concourse.bass as bass
    import concourse.tile as tile
    import concourse.mybir as mybir
    from concourse.masks import make_identity
    from contextlib import ExitStack

    f32 = mybir.dt.float32
    f16 = mybir.dt.float16
    bf = mybir.dt.bfloat16
    i8 = mybir.dt.int8
    u8 = mybir.dt.uint8
    MULT = mybir.AluOpType.mult
    ADD = mybir.AluOpType.add
    SUB = mybir.AluOpType.subtract
    ISEQ = mybir.AluOpType.is_equal
    MAX = mybir.AluOpType.max
    RND = 12582912.0    # 1.5*2^23: (x + RND) - RND rounds x to nearest int
    Exp = mybir.ActivationFunctionType.Exp
    Sqrt = mybir.ActivationFunctionType.Sqrt
    AX = mybir.AxisListType.X

    N = H * W
    NB = H // RB
    assert H % RB == 0
    NT = RB * W // 128          # pixel-tiles per block (16 at full size)
    scale = HD ** -0.5

    nc = bacc.Bacc("TRN2", target_bir_lowering=False, debug=False,
                   num_devices=n_cores)

    x_d = nc.dram_tensor("x", [C, N], i8, kind="ExternalInput")
    xs_d = nc.dram_tensor("xs", [C, 1], f32, kind="ExternalInput")
    # gates packed 2 bytes/pixel: row 0 = i1*8+i2 (host-exact top-2 head
    # indices), row 1 = round((g1*TOPK - 1)*255) with g1 the larger gate;
    # g2*TOPK = 2 - g1*TOPK exactly
    g_d = nc.dram_tensor("g", [2, N], u8, kind="ExternalInput")
    wA_d = nc.dram_tensor("wA", [C, 576], f16, kind="ExternalInput")
    dwd_d = nc.dram_tensor("dwd", [128, 45, 128], bf, kind="ExternalInput")
    pj_d = nc.dram_tensor("pj", [C, C], bf, kind="ExternalInput")
    NCH = N // 512
    # int8 output with per-(row, 128px-chunk) scales: halves d2h bytes; the
    # chunk absmax scale keeps quantization noise <1% of chunk rms
    outq_d = nc.dram_tensor("outq", [C, N], i8, kind="ExternalOutput")
    outs_d = nc.dram_tensor("outs", [96, 2 * NCH * 4], f16, kind="ExternalOutput")
    if dbg:
        dbg_v0 = nc.dram_tensor("dbg_v0", [96, N], f32, kind="ExternalOutput")
        dbg_qk0 = nc.dram_tensor("dbg_qk0", [96, N], f32, kind="ExternalOutput")
        dbg_gram = nc.dram_tensor("dbg_gram", [96, 384], f32, kind="ExternalOutput")
        dbg_bd = nc.dram_tensor("dbg_bd", [96, 192], f32, kind="ExternalOutput")
        dbg_pad0 = nc.dram_tensor("dbg_pad0", [128, (RB + 2) * (W + 2)], f32,
                                  kind="ExternalOutput")

    # conv output channel chunks: 4x128 qkv + 64 v-tail
    OCS = [(0, 128), (128, 128), (256, 128), (384, 128), (512, 64)]
    # dwconv channel chunks ( = pad buffers )
    DWS = [128, 128, 128, 128, 64]
    PADW = W + 2
    PADF = (RB + 2) * PADW

    with ExitStack() as top:
        tc = top.enter_context(tile.TileContext(nc))
        singles = top.enter_context(tc.tile_pool(name="singles", bufs=1))

        # --- resident constants ---
        wA0 = singles.tile([96, 576], f16)
        wA1 = singles.tile([96, 576], f16)
        nc.sync.dma_start(wA0[:], wA_d[0:96, :])
        nc.sync.dma_start(wA1[:], wA_d[96:192, :])
        dwd = singles.tile([128, 45, 128], bf)
        nc.sync.dma_start(dwd[:], dwd_d[:])
        xst = singles.tile([96, 2], f32)    # x dequant scales, rows 0..95/96..191
        nc.sync.dma_start(xst[:, 0:1], xs_d[0:96, :])
        nc.sync.dma_start(xst[:, 1:2], xs_d[96:192, :])
        iota8 = singles.tile([8, 1], f32)   # 0..7 column for gate-index decode
        iotai = singles.tile([8, 1], mybir.dt.int32)
        nc.gpsimd.iota(iotai[:], [[1, 1]], channel_multiplier=1)
        nc.vector.tensor_copy(iota8[:], iotai[:])
        ident = singles.tile([128, 128], f32)
        make_identity(nc, ident[:])
        identb = singles.tile([128, 128], bf)
        nc.vector.tensor_copy(identb[:], ident[:])
        pjt = singles.tile([96, 2, 2, 96], bf)   # [c-half, o-half][96c, 96o]
        for ch in range(2):
            for oh in range(2):
                nc.sync.dma_start(pjt[:, ch, oh, :],
                                  pj_d[96 * ch:96 * ch + 96, 96 * oh:96 * oh + 96])

        # --- resident accumulators / outputs of pass 1 ---
        v0 = singles.tile([96, N], bf)       # gated v, channels 0..95
        v1 = singles.tile([96, N], bf)       # gated v, channels 96..191
        gacc = singles.tile([96, 2, 192], f32)  # gram accumulators (4 groups)

        p1 = top.enter_context(ExitStack())
        xp = p1.enter_context(tc.tile_pool(name="xp", bufs=2))
        padp = p1.enter_context(tc.tile_pool(name="padp", bufs=1))
        qkp = p1.enter_context(tc.tile_pool(name="qkp", bufs=1))
        gep = p1.enter_context(tc.tile_pool(name="gep", bufs=1))
        stp = p1.enter_context(tc.tile_pool(name="stp", bufs=2))
        ps_conv = p1.enter_context(tc.tile_pool(name="ps_conv", bufs=1, space="PSUM"))
        ps_dw = p1.enter_context(tc.tile_pool(name="ps_dw", bufs=1, space="PSUM"))
        ps_tp = p1.enter_context(tc.tile_pool(name="ps_tp", bufs=1, space="PSUM"))
        ps_gr = p1.enter_context(tc.tile_pool(name="ps_gr", bufs=1, space="PSUM"))

        for b in range(NB):
            r0 = b * RB
            lo = max(r0 - 1, 0)              # first conv'd image row
            hi = min(r0 + RB + 1, H)         # one past last conv'd image row
            span = hi - lo                    # 16+1/2 rows incl halos
            spx = span * W

            # --- load x rows [lo, hi) as int8, dequantize to f16 ---
            xi0 = xp.tile([96, (RB + 2) * W], i8, tag="xi0")
            xi1 = xp.tile([96, (RB + 2) * W], i8, tag="xi1")
            nc.sync.dma_start(xi0[:, 0:spx], x_d[0:96, lo * W:hi * W])
            nc.sync.dma_start(xi1[:, 0:spx], x_d[96:192, lo * W:hi * W])
            xb0 = xp.tile([96, (RB + 2) * W], f16, tag="xb0")
            xb1 = xp.tile([96, (RB + 2) * W], f16, tag="xb1")
            nc.vector.tensor_scalar(xb0[:, 0:spx], xi0[:, 0:spx],
                                    xst[:, 0:1], None, op0=MULT)
            nc.vector.tensor_scalar(xb1[:, 0:spx], xi1[:, 0:spx],
                                    xst[:, 1:2], None, op0=MULT)

            # --- pad buffers for dwconv input ---
            pads = [padp.tile([DWS[i], (RB + 2), PADW], bf, tag=f"pad{i}",
                              name=f"pad{i}") for i in range(5)]
            for i, pd in enumerate(pads):
                nc.vector.memset(pd[:, :, 0:1], 0)
                nc.vector.memset(pd[:, :, PADW - 1:PADW], 0)
                if b == 0:
                    nc.vector.memset(pd[:, 0:1, :], 0)
                if b == NB - 1:
                    nc.vector.memset(pd[:, RB + 1:RB + 2, :], 0)

            # --- conv1x1: chunks over the conv span ---
            chunks = []
            p0 = 0
            while p0 < spx:
                sz = min(512, spx - p0)
                chunks.append((p0, sz))
                p0 += sz
            for (p0, sz) in chunks:
                s_a = p0 // W + (1 if b == 0 else 0)   # pad-row of chunk start
                nrows = sz // W
                for oi, (ob, osz) in enumerate(OCS):
                    pc = ps_conv.tile([128, 512], f32, tag="pc")
                    mm = pc[0:osz, 0:sz]
                    nc.tensor.matmul(mm, wA0[:, ob:ob + osz], xb0[:, p0:p0 + sz],
                                     start=True, stop=False)
                    nc.tensor.matmul(mm, wA1[:, ob:ob + osz], xb1[:, p0:p0 + sz],
                                     start=False, stop=True)
                    src3 = pc[0:osz, 0:sz].rearrange("c (r w) -> c r w", w=W)
                    dst = pads[oi][:, s_a:s_a + nrows, 1:W + 1]
                    nc.any.tensor_copy(dst, src3)

            # --- host-packed gates: decode 2B/pixel -> [8, BW] bf16, replicate ---
            BW = RB * W
            gpk0 = gep.tile([1, BW], u8, tag="gpk0")
            gpk1 = gep.tile([1, BW], u8, tag="gpk1")
            nc.sync.dma_start(gpk0[:], g_d[0:1, r0 * W:(r0 + RB) * W])
            nc.sync.dma_start(gpk1[:], g_d[1:2, r0 * W:(r0 + RB) * W])
            bu0 = gep.tile([8, BW], u8, tag="bu0")
            bu1 = gep.tile([8, BW], u8, tag="bu1")
            nc.gpsimd.partition_broadcast(bu0[:], gpk0[:])
            nc.gpsimd.partition_broadcast(bu1[:], gpk1[:])
            bidx = gep.tile([8, BW], f32, tag="bidx")
            bg1 = gep.tile([8, BW], f32, tag="bg1")
            nc.vector.tensor_copy(bidx[:], bu0[:])
            nc.vector.tensor_scalar_mul(bg1[:], bu1[:], 1.0 / 255.0)
            # i1 = round(idx/8 - 0.4375); i2 = idx - 8*i1  (exact small ints)
            i1t = gep.tile([8, BW], f32, tag="i1t")
            nc.vector.tensor_scalar(i1t[:], bidx[:], 0.125, -0.4375,
                                    op0=MULT, op1=ADD)
            nc.vector.tensor_scalar(i1t[:], i1t[:], RND, -RND,
                                    op0=ADD, op1=ADD)
            i2t = gep.tile([8, BW], f32, tag="i2t")
            nc.vector.tensor_scalar(i2t[:], i1t[:], -8.0, None, op0=MULT)
            nc.vector.tensor_tensor(out=i2t[:], in0=i2t[:], in1=bidx[:], op=ADD)
            # e1/e2 in place of i1t/i2t; gates = e1 + e2 + (u/255)*(e1 - e2)
            nc.vector.tensor_scalar(i1t[:], i1t[:], iota8[:, 0:1], None, op0=ISEQ)
            nc.vector.tensor_scalar(i2t[:], i2t[:], iota8[:, 0:1], None, op0=ISEQ)
            nc.vector.tensor_tensor(out=bidx[:], in0=i1t[:], in1=i2t[:], op=SUB)
            nc.vector.tensor_tensor(out=bidx[:], in0=bidx[:], in1=bg1[:], op=MULT)
            nc.vector.tensor_tensor(out=bidx[:], in0=bidx[:], in1=i1t[:], op=ADD)
            gA = gep.tile([8, BW], bf, tag="gA")
            nc.vector.tensor_tensor(out=gA[:], in0=bidx[:], in1=i2t[:], op=ADD)
            gx0 = gep.tile([96, BW], bf, tag="gx0")   # heads 0..3 x24
            gx1 = gep.tile([96, BW], bf, tag="gx1")   # heads 4..7 x24
            s0 = bass.AP(tensor=gA.tensor, offset=gA[:].offset,
                         ap=[[BW, 4], [0, 24], [1, BW]])
            s1 = bass.AP(tensor=gA.tensor, offset=gA[4:8, :].offset,
                         ap=[[BW, 4], [0, 24], [1, BW]])
            nc.sync.dma_start(gx0[:], s0)
            nc.sync.dma_start(gx1[:], s1)

            # --- depthwise conv 3x3 + v gating ---
            qk = [qkp.tile([96, RB * W], bf, tag=f"qk{g}", name=f"qk{g}")
                  for g in range(4)]
            nch = RB * W // 512
            for ci in range(5):
                csz = DWS[ci]
                for u in range(nch):
                    pd = ps_dw.tile([128, 512], f32, tag="pd")
                    y0 = (u * 512) // W          # interior row offset 0..RB-1
                    nr = 512 // W
                    for t in range(9):
                        dy, dx = t // 3 - 1, t % 3 - 1
                        rhs = pads[ci][:, y0 + 1 + dy:y0 + 1 + dy + nr,
                                       1 + dx:1 + dx + W]
                        nc.tensor.matmul(
                            pd[0:csz, :].rearrange("c (r w) -> c r w", w=W),
                            dwd[0:csz, 5 * t + ci, 0:csz], rhs,
                            start=(t == 0), stop=(t == 8))
                    # NOTE: SBUF operands must start at partition {0,32,64,96}
                    # with span <= {128,32,64,32}; PSUM sources are exempt.
                    sl = slice(u * 512, (u + 1) * 512)
                    glob = slice(r0 * W + u * 512, r0 * W + (u + 1) * 512)
                    if ci == 0:
                        nc.any.tensor_copy(qk[0][0:96, sl], pd[0:96, :])
                        nc.any.tensor_copy(qk[1][0:32, sl], pd[96:128, :])
                    elif ci == 1:
                        nc.any.tensor_copy(qk[1][32:64, sl], pd[0:32, :])
                        nc.any.tensor_copy(qk[1][64:96, sl], pd[32:64, :])
                        nc.any.tensor_copy(qk[2][0:64, sl], pd[64:128, :])
                    elif ci == 2:
                        nc.any.tensor_copy(qk[2][64:96, sl], pd[0:32, :])
                        nc.any.tensor_copy(qk[3][0:32, sl], pd[32:64, :])
                        nc.any.tensor_copy(qk[3][32:64, sl], pd[64:96, :])
                        nc.any.tensor_copy(qk[3][64:96, sl], pd[96:128, :])
                    elif ci == 3:
                        nc.vector.tensor_tensor(out=v0[:, glob], in0=pd[0:96, :],
                                                in1=gx0[:, sl], op=MULT)
                        nc.vector.tensor_tensor(out=v1[0:32, glob],
                                                in0=pd[96:128, :],
                                                in1=gx1[0:32, sl], op=MULT)
                    else:
                        nc.vector.tensor_tensor(out=v1[32:64, glob],
                                                in0=pd[0:32, :],
                                                in1=gx1[32:64, sl], op=MULT)
                        nc.vector.tensor_tensor(out=v1[64:96, glob],
                                                in0=pd[32:64, :],
                                                in1=gx1[64:96, sl], op=MULT)

            # --- q/k pixel-tile transposes + gram accumulation ---
            grp = [ps_gr.tile([96, 96], f32, tag=f"gr{g}", name=f"gr{g}")
                   for g in range(4)]
            for j in range(NT):
                st = stp.tile([128, 4, 4, 24], bf, tag="st")  # [p, gp, slot, hd]
                for g in range(4):
                    tq = ps_tp.tile([128, 96], bf, tag="tq")
                    nc.tensor.transpose(tq[:], qk[g][:, j * 128:(j + 1) * 128],
                                        identb[0:96, 0:96])
                    src = tq[:].rearrange("p (a b h) -> p a b h", a=2, b=2, h=24)
                    if g == 0:
                        nc.any.tensor_copy(st[:, 0:2, 0:2, :], src)
                    elif g == 1:
                        nc.any.tensor_copy(st[:, 2:4, 0:2, :], src)
                    elif g == 2:
                        nc.any.tensor_copy(st[:, 0:2, 2:4, :], src)
                    else:
                        nc.any.tensor_copy(st[:, 2:4, 2:4, :], src)
                for gp in range(4):
                    lhs = st[:, gp, :, :].rearrange("p a b -> p (a b)")
                    nc.tensor.matmul(grp[gp], lhs, lhs,
                                     start=(j == 0), stop=(j == NT - 1))
            if dbg == 2 and b == 0:
                dp0 = qkp.tile([128, (RB + 2) * PADW], f32, tag="dp0")
                nc.vector.tensor_copy(dp0[:], pads[0][:].rearrange("c a b -> c (a b)"))
                nc.sync.dma_start(dbg_pad0[:], dp0[:])
            if dbg == 2:
                dv0 = qkp.tile([96, RB * W], f32, tag="dv0")
                nc.vector.tensor_copy(dv0[:], v0[:, r0 * W:(r0 + RB) * W])
                nc.sync.dma_start(dbg_v0[:, r0 * W:(r0 + RB) * W], dv0[:])
                dqk = qkp.tile([96, RB * W], f32, tag="dqk")
                nc.vector.tensor_copy(dqk[:], qk[0][:, 0:RB * W])
                nc.sync.dma_start(dbg_qk0[:, r0 * W:(r0 + RB) * W], dqk[:])
            for gp in range(4):
                dstg = gacc[:, gp // 2, (gp % 2) * 96:(gp % 2) * 96 + 96]
                if b == 0:
                    nc.any.tensor_copy(dstg, grp[gp])
                else:
                    nc.vector.tensor_tensor(out=dstg, in0=dstg, in1=grp[gp], op=ADD)
        p1.close()

        # ===== pass 2: attention matrices =====
        p2 = top.enter_context(ExitStack())
        smp = p2.enter_context(tc.tile_pool(name="smp", bufs=1))
        dramp = p2.enter_context(tc.tile_pool(name="dramp", bufs=1, space="DRAM"))
        # assemble block-diag attn in DRAM (partition-offset bf16 SBUF DMA
        # writes drop elements on HW), then load+convert once
        bd_dram = dramp.tile([96, 2, 96], f32)
        zst = smp.tile([96, 2, 96], f32, name="zst")
        nc.vector.memset(zst[:], 0)
        nc.sync.dma_start(bd_dram[:], zst[:])

        bd = [singles.tile([96, 96], bf, name="bd0"),
              singles.tile([96, 96], bf, name="bd1")]
        nc.vector.memset(bd[0][:], 0)
        nc.vector.memset(bd[1][:], 0)

        rinv = smp.tile([96, 4], f32)
        for gp in range(4):
            G = gacc[:, gp // 2, (gp % 2) * 96:(gp % 2) * 96 + 96]
            dt_ = smp.tile([96, 96], f32, tag="dt_")
            nc.vector.tensor_tensor(out=dt_[:], in0=G, in1=ident[0:96, 0:96],
                                    op=MULT)
            ssq = smp.tile([96, 1], f32, tag="ssq")
            nc.vector.tensor_reduce(ssq[:], dt_[:], axis=AX, op=ADD)
            nc.scalar.activation(ssq[:], ssq[:], Sqrt)
            nc.vector.tensor_scalar_max(ssq[:], ssq[:], 1e-12)
            nc.vector.reciprocal(rinv[:, gp:gp + 1], ssq[:])

        for gp in range(4):
            G = gacc[:, gp // 2, (gp % 2) * 96:(gp % 2) * 96 + 96]
            for m in range(2):
                h = 2 * gp + m
                # 24-row-aligned slices are illegal SBUF operands -> stage
                # through SBUF->SBUF DMA into partition-0-based tiles.
                gblk = smp.tile([24, 24], f32, tag="gblk")
                nc.sync.dma_start(gblk[:],
                                  G[24 * m:24 * m + 24, 48 + 24 * m:72 + 24 * m])
                rq = smp.tile([24, 1], f32, tag="rq")
                nc.sync.dma_start(rq[:], rinv[24 * m:24 * m + 24, gp:gp + 1])
                # k-norm column -> row via 32x32 DVE transpose
                zt = smp.tile([32, 32], f32, tag="zt")
                nc.vector.memset(zt[:], 0)
                nc.sync.dma_start(zt[0:24, 0:1],
                                  rinv[48 + 24 * m:72 + 24 * m, gp:gp + 1])
                ztt = smp.tile([32, 32], f32, tag="ztt")
                nc.vector.transpose(ztt[:], zt[:])
                O = smp.tile([24, 24], f32, tag="O")
                nc.gpsimd.partition_broadcast(O[:], ztt[0:1, 0:24])
                nc.vector.tensor_scalar(O[:], O[:], rq[:],
                                        float(scale), op0=MULT, op1=MULT)
                al32 = smp.tile([32, 32], f32, tag="al32")
                nc.vector.memset(al32[:], 0)
                al = al32[0:24, 0:24]
                nc.vector.tensor_tensor(out=al, in0=gblk[:], in1=O[:], op=MULT)
                negm = smp.tile([24, 1], f32, tag="negm")
                nc.vector.tensor_reduce(negm[:], al, axis=AX,
                                        op=mybir.AluOpType.max, negate=True)
                den = smp.tile([24, 1], f32, tag="den")
                nc.scalar.activation(al, al, Exp, bias=negm[:],
                                     accum_out=den[:])
                rden = smp.tile([24, 1], f32, tag="rden")
                nc.vector.reciprocal(rden[:], den[:])
                nc.vector.tensor_scalar(al, al, rden[:], None, op0=MULT)
                patv = smp.tile([32, 32], f32, tag="patv")
                nc.vector.transpose(patv[:], al32[:])
                sa = smp.tile([24, 24], f32, tag="sa")
                nc.any.tensor_copy(sa[:], patv[0:24, 0:24])
                hh = h % 4
                nc.sync.dma_start(bd_dram[24 * hh:24 * hh + 24, h // 4,
                                          24 * hh:24 * hh + 24], sa[:])
        bdf = smp.tile([96, 2, 96], f32, name="bdf")
        nc.sync.dma_start(bdf[:], bd_dram[:])
        nc.any.tensor_copy(bd[0][:], bdf[:, 0, :])
        nc.any.tensor_copy(bd[1][:], bdf[:, 1, :])
        if dbg:
            nc.sync.dma_start(dbg_gram[:], gacc[:].rearrange("p a b -> p (a b)"))
            dbd = smp.tile([96, 192], f32, name="dbd")
            nc.vector.tensor_copy(dbd[:, 0:96], bd[0][:])
            nc.vector.tensor_copy(dbd[:, 96:192], bd[1][:])
            nc.sync.dma_start(dbg_bd[:], dbd[:])
        p2.close()

        # ===== pass 3: attn @ v_gated, proj, int8 quantize, out =====
        p3 = top.enter_context(ExitStack())
        op_ = p3.enter_context(tc.tile_pool(name="op_", bufs=3))
        oqp = p3.enter_context(tc.tile_pool(name="oqp", bufs=1))
        ps3 = p3.enter_context(tc.tile_pool(name="ps3", bufs=2, space="PSUM"))
        q8 = [oqp.tile([96, N], i8, name="q80"),
              oqp.tile([96, N], i8, name="q81")]
        scol = oqp.tile([96, 2, NCH, 4], f32)
        for u in range(NCH):
            sl = slice(u * 512, (u + 1) * 512)
            avs = []
            for half in range(2):
                pav = ps3.tile([96, 512], f32, tag=f"pav{half}")
                nc.tensor.matmul(pav[:], bd[half][:], (v0 if half == 0 else v1)[:, sl],
                                 start=True, stop=True)
                av = op_.tile([96, 512], bf, tag=f"av{half}")
                nc.any.tensor_copy(av[:], pav[:])
                avs.append(av)
            for oh in range(2):
                po = ps3.tile([96, 512], f32, tag=f"po{oh}")
                nc.tensor.matmul(po[:], pjt[:, 0, oh, :], avs[0][:],
                                 start=True, stop=False)
                nc.tensor.matmul(po[:], pjt[:, 1, oh, :], avs[1][:],
                                 start=False, stop=True)
                po3 = po[:].rearrange("c (a b) -> c a b", b=128)
                av = scol[:, oh, u, :]                            # [96, 4]
                av3 = av.rearrange("c (a o) -> c a o", o=1)
                nc.vector.tensor_reduce(av3, po3, axis=AX, op=MAX,
                                        apply_absolute_value=True)
                nc.vector.tensor_scalar_max(av, av, 1e-30)
                rcl = op_.tile([96, 4], f32, tag=f"rcl{oh}")
                nc.vector.reciprocal(rcl[:], av)
                nc.vector.tensor_scalar_mul(rcl[:], rcl[:], 127.0)
                rclb = rcl[:].rearrange("c (a o) -> c a o", o=1) \
                             .to_broadcast([96, 4, 128])
                qf = op_.tile([96, 512], f32, tag=f"qf{oh}")
                qf3 = qf[:].rearrange("c (a b) -> c a b", b=128)
                nc.vector.tensor_tensor(out=qf3, in0=po3, in1=rclb, op=MULT)
                nc.vector.tensor_scalar(qf[:], qf[:], RND, -RND,
                                        op0=ADD, op1=ADD)
                nc.any.tensor_copy(q8[oh][:, sl], qf[:])
        nc.sync.dma_start(outq_d[0:96, :], q8[0][:])
        nc.sync.dma_start(outq_d[96:192, :], q8[1][:])
        scol16 = op_.tile([96, 2 * NCH * 4], f16, name="scol16")
        nc.vector.tensor_copy(scol16[:], scol[:].rearrange("p a b c -> p (a b c)"))
        nc.sync.dma_start(outs_d[:], scol16[:])
        p3.close()

    nc.finalize()
    return nc


def _host_prep(qkv_w, dw_w, proj_w):
    wA = np.ascontiguousarray(qkv_w.T).astype(np.float16)  # [192, 576]
    w9 = dw_w.reshape(3 * C, 9).astype(np.float32)
    dwd = np.zeros((128, 45, 128), dtype=ml_dtypes.bfloat16)
    DWS = [128, 128, 128, 128, 64]
    for t in range(9):
        for i in range(5):
            base = sum(DWS[:i])
            csz = DWS[i]
            m = np.zeros((128, 128), np.float32)
            np.fill_diagonal(m[:csz, :csz], w9[base:base + csz, t])
            dwd[:, 5 * t + i, :] = m.astype(ml_dtypes.bfloat16)
    pj = np.ascontiguousarray(proj_w.T).astype(ml_dtypes.bfloat16)  # [192c, 192o]
    return wA, dwd, pj


def _host_gates(x2, rw):
    """Exact fp32 router on host: x2 [B, C, N], rw [8, C] -> packed gates
    [B, 2, N] uint8: row 0 = i1*8+i2 (top-2 head indices), row 1 =
    round((g1*TOPK - 1)*255) with g1 the larger renormalized gate."""
    B = x2.shape[0]
    N = x2.shape[2]
    logits = np.matmul(rw[None].astype(np.float32), x2)          # [B, 8, N]
    m = logits.max(axis=1, keepdims=True)
    p = np.exp(logits - m)
    p /= p.sum(axis=1, keepdims=True)
    i1 = p.argmax(axis=1)                                        # [B, N]
    p1 = np.take_along_axis(p, i1[:, None], axis=1)[:, 0]
    np.put_along_axis(p, i1[:, None], -1.0, axis=1)
    i2 = p.argmax(axis=1)
    p2 = np.take_along_axis(p, i2[:, None], axis=1)[:, 0]
    g1t = (float(TOPK) * p1) / np.maximum(p1 + p2, 1.1920929e-7)  # in [1, 2]
    out = np.empty((B, 2, N), np.uint8)
    out[:, 0] = (i1 * 8 + i2).astype(np.uint8)
    out[:, 1] = np.rint((g1t - 1.0) * 255.0).astype(np.uint8)
    return out


class _State:
    pass


def _get_state(H, W, B):
    key = (H, W, B)
    if key in _CACHE:
        return _CACHE[key]
    import jax
    import jax.numpy as jnp
    import concourse.mybir as mybir
    from concourse.bass2jax import (_bass_exec_p, partition_id_tensor,
                                    install_neuronx_cc_hook)

    install_neuronx_cc_hook()
    nc = _build(H, W, 16, 1)

    partition_name = (nc.partition_id_tensor.name
                      if nc.partition_id_tensor else None)
    in_names, out_names, out_avals = [], [], []
    for alloc in nc.m.functions[0].allocations:
        if not isinstance(alloc, mybir.MemoryLocationSet):
            continue
        name = alloc.memorylocations[0].name
        if alloc.kind == "ExternalInput":
            if name != partition_name:
                in_names.append(name)
        elif alloc.kind == "ExternalOutput":
            out_names.append(name)
            out_avals.append(jax.core.ShapedArray(
                tuple(alloc.tensor_shape), mybir.dt.np(alloc.dtype)))
    n_params = len(in_names)
    all_names = tuple(in_names + out_names +
                      ([partition_name] if partition_name else []))
    donate = tuple(range(n_params, n_params + len(out_names)))

    def _body(*args):
        operands = list(args)
        if partition_name is not None:
            operands.append(partition_id_tensor())
        return tuple(_bass_exec_p.bind(
            *operands,
            out_avals=tuple(out_avals),
            in_names=all_names,
            out_names=tuple(out_names),
            lowering_input_output_aliases=(),
            sim_require_finite=True,
            sim_require_nnan=True,
            nc=nc,
        ))

    st = _State()
    st.nc = nc
    st.in_names = in_names
    st.out_names = out_names
    st.out_avals = out_avals
    st.jit_body = jax.jit(_body, donate_argnums=donate, keep_unused=True)
    st.devices = jax.devices()[:B]
    # donated output scratch buffers, one per device; replaced by each call's
    # result (the kernel writes every element of out, so no pre-zero needed)
    zeros = [np.zeros(tuple(a.shape), a.dtype) for a in out_avals]
    st.scratch = [[jax.device_put(z, d) for z in zeros] for d in st.devices]
    st.weights_fp = None
    st.wdev = None
    _CACHE[key] = st
    return st


def kernel(x, qkv_w, dw_w, proj_w, router_main_w, router_aux_w, task_id):
    import jax

    x = np.asarray(x, np.float32)
    B, c, H, W = x.shape
    assert c == C
    N = H * W
    tid = int(np.asarray(task_id))
    rw = np.asarray(router_main_w if tid == 0 else router_aux_w, np.float32)

    st = _get_state(H, W, B)

    # --- weights: content-hashed, uploaded to each device only on change
    # (rw excluded: the router runs on host, device weights don't use it) ---
    hsh = hashlib.blake2b(digest_size=16)
    for a in (qkv_w, dw_w, proj_w):
        hsh.update(np.ascontiguousarray(a, np.float32).tobytes())
    wfp = hsh.digest()
    if st.weights_fp != wfp:
        wA, dwd, pj = _host_prep(np.asarray(qkv_w, np.float32),
                                 np.asarray(dw_w, np.float32),
                                 np.asarray(proj_w, np.float32))
        wmap = {"wA": wA, "dwd": dwd, "pj": pj}
        st.wdev = [{k: jax.device_put(v, d) for k, v in wmap.items()}
                   for d in st.devices]
        st.weights_fp = wfp

    x2 = x.reshape(B, C, N)

    # --- dispatch per core, interleaving host conversion with the wire so
    # transfers start immediately; h2d / exec / d2h pipeline over the tunnel ---
    results = []
    for b in range(B):
        dev = st.devices[b]
        am = np.abs(x2[b]).max(axis=1, keepdims=True)         # [C, 1]
        qs = 127.0 / np.maximum(am, 1e-30)
        xq = np.rint(x2[b] * qs).astype(np.int8)
        per = {"x": jax.device_put(xq, dev),
               "xs": jax.device_put((am / 127.0).astype(np.float32), dev),
               "g": jax.device_put(_host_gates(x2[b:b + 1], rw)[0], dev)}
        ops = [per[name] if name in per else st.wdev[b][name]
               for name in st.in_names]
        res = st.jit_body(*ops, *st.scratch[b])
        for r in res:
            r.copy_to_host_async()
        results.append(res)

    qi = st.out_names.index("outq")
    si = st.out_names.index("outs")
    NQ = N // 128
    out = np.empty((B, C, N), np.float32)
    for b in range(B):
        q = np.asarray(results[b][qi])               # [C, N] int8
        s = np.asarray(results[b][si]).astype(np.float32)   # [96, 2*NQ] f16
        sc = s.reshape(96, 2, NQ).transpose(1, 0, 2).reshape(C, NQ) / 127.0
        np.multiply(q.reshape(C, NQ, 128).astype(np.float32), sc[:, :, None],
                    out=out[b].reshape(C, NQ, 128))
    st.scratch = [list(res) for res in results]
    return out.reshape(B, C, H, W)
